# revision 12
# baseline (speedup 1.0000x reference)
"""Trainium2 Bass kernel for nn_CumulativeFlattenedLinear (segment_reduce).

Computation: per window of S=64 timesteps, per-timestep C->O linear projection
(weights zero for the first n_discard steps) followed by a causal cumsum within
the window, plus bias.

Strategy (data-parallel over batch, 1 batch element per core):
  - Per 8-step sub-block u: a triangular-masked "intra" matmul plus a "prefix"
    matmul targeting later sub-blocks; both issued as ONE stacked N=256 bf16
    matmul writing [intra | pre] contiguously in PSUM (pre region shared per
    u-pair, accumulated in PSUM).
  - x is loaded with partition = 256-element time chunk (1KB contiguous DMA
    runs, split in two half-tile DMAs so compute starts early), shuffled+cast
    to bf16 in (u, c, v') column order on the Scalar engine, transposed
    128x128 on the TensorEngine (bf16: 1 cyc/row), copied PSUM->SBUF (Scalar).
  - prefix totals summed across the 3 pair-regions + bias (DVE), then one
    strided combine per window writes the (o, t)-ordered output tile; the
    bias-only head region (s < n_discard) is filled once per supertile (DVE).
  - Output stored in two half-tile DMAs per supertile (Scalar queue) so the
    store stream overlaps the input stream; constants load on the GpSimd
    queue so the first x tile is the first packet on the input queue.
"""
import numpy as np
import ml_dtypes

import concourse.bass as bass
import concourse.tile as tile
from concourse import bacc, mybir
from concourse.bass_utils import run_bass_kernel_spmd

F32 = mybir.dt.float32
BF16 = mybir.dt.bfloat16

# problem geometry (asserted against inputs at runtime)
B, C, T, O = 8, 16, 131072, 16
P = 128
CH = 256                 # time-elements per partition per supertile
NST = T // (P * CH)      # 4 supertiles
V = 8                    # sub-block length
NU = 8                   # sub-blocks per window

_cache = {}


def _build_nc(du_count):
    """Build the per-core Bass program. du_count = number of active sub-blocks
    (those with any nonzero weight), assumed to be the trailing ones."""
    S = NU * V  # 64
    NW = CH // S  # windows per partition = 4
    DU = du_count
    first_u = NU - DU          # first active sub-block
    fill_s = first_u * V       # s < fill_s -> output = bias

    nc = bacc.Bacc("TRN2", target_bir_lowering=False, debug=False)
    x_d = nc.dram_tensor("x", (C, T), F32, kind="ExternalInput")
    w_d = nc.dram_tensor("w_all", (P, DU * 256), BF16, kind="ExternalInput")
    bpre_d = nc.dram_tensor("bias_pre", (P, P), F32, kind="ExternalInput")
    ident_d = nc.dram_tensor("ident", (P, P), BF16, kind="ExternalInput")
    seed_d = nc.dram_tensor("seed", (1, 2 * P), BF16, kind="ExternalInput")
    y_d = nc.dram_tensor("y", (O, T), F32, kind="ExternalOutput")

    xv = x_d.ap().rearrange("c (st p hs) -> st p c hs", st=NST, p=P, hs=CH)
    yv = y_d.ap().rearrange("o (st p hs) -> st p o hs", st=NST, p=P, hs=CH)

    NB = (DU + 1) // 2  # psum banks per window group
    H = CH // 2         # half-tile split for load/store DMAs

    with tile.TileContext(nc) as tc:
        with (
            tc.tile_pool(name="const", bufs=1) as cp,
            tc.tile_pool(name="io", bufs=3) as io,
            tc.tile_pool(name="mid", bufs=3) as mid,
            tc.tile_pool(name="psW", bufs=2, space="PSUM") as psW,
            tc.tile_pool(name="psT", bufs=2, space="PSUM") as psT,
        ):
            # constants on the Scalar DMA queue (idle until the first store):
            # the Sync queue's first descriptors are then the first x tile.
            w_all = cp.tile([P, DU * 256], BF16, name="w_all")
            nc.scalar.dma_start(w_all[:], w_d.ap())
            bias_pre = cp.tile([P, P], F32, name="bias_pre")
            nc.scalar.dma_start(bias_pre[:], bpre_d.ap())
            ident = cp.tile([P, P], BF16, name="ident")
            nc.scalar.dma_start(ident[:], ident_d.ap())
            # seed row: [ones(128) | bias tiled over (u,o)] on one partition
            seed = cp.tile([1, 2 * P], BF16, name="seed")
            nc.scalar.dma_start(seed[:], seed_d.ap())

            for st in range(NST):
                xin = io.tile([P, C * CH], F32, name="xin", tag="xin")
                if st == 0:
                    # split the first load so window-group 0 starts early
                    xin_v = xin[:].rearrange("p (c hs) -> p c hs", c=C)
                    nc.sync.dma_start(xin_v[:, :, 0:H], xv[st][:, :, 0:H])
                    nc.sync.dma_start(xin_v[:, :, H:CH], xv[st][:, :, H:CH])
                else:
                    nc.sync.dma_start(
                        xin[:].rearrange("p (c hs) -> p c hs", c=C), xv[st]
                    )
                out_sb = io.tile([P, O * CH], F32, name="out_sb", tag="out")
                # ---- bias fill for s < fill_s, all windows of this st ----
                if fill_s:
                    outf = out_sb[:].rearrange(
                        "p (o w u v) -> p o w u v", o=O, w=NW, u=NU, v=V
                    )[:, :, :, 0:first_u]
                    bsrc = (
                        bias_pre[:, 0:O]
                        .unsqueeze(2).unsqueeze(3).unsqueeze(4)
                        .broadcast_to([P, O, NW, first_u, V])
                    )
                    nc.gpsimd.tensor_copy(outf, bsrc)
                for wdw in range(NW):
                    # ---- shuffle+cast to (du, c, v) bf16 (Scalar) ----
                    shuf = mid.tile([P, DU * 128], BF16, name="shuf", tag="shuf")
                    src = xin[:].rearrange(
                        "p (c w u v) -> w p u c v", c=C, w=NW, u=NU, v=V
                    )[wdw, :, first_u:NU]
                    nc.scalar.copy(
                        shuf[:].rearrange("p (u c v) -> p u c v", u=DU, c=C, v=V),
                        src,
                    )
                    # ---- transposes (PE): all DU into one bf16 psum bank ----
                    pt = psT.tile([P, DU * 128], BF16, name="pt", tag="pt")
                    for j in range(DU):
                        nc.tensor.transpose(
                            pt[:, j * 128:(j + 1) * 128],
                            shuf[:, j * 128:(j + 1) * 128],
                            ident[:],
                            tile_position=(0, 0),
                        )
                    ts = mid.tile([P, DU * 128], BF16, name="ts", tag="ts")
                    nc.vector.tensor_copy(ts[:], pt[:])
                    # ---- matmuls; bank0's pre region is seeded with bias ----
                    pw = psW.tile([P, NB * 512], F32, name="pw", tag="pw")
                    nc.tensor.matmul(
                        pw[:, 128:256], seed[:, 0:P], seed[:, P:2 * P],
                        start=True, stop=False, skip_group_check=True,
                    )
                    for du in range(DU):
                        bk = du // 2
                        lo = bk * 512 + (du % 2) * 128
                        nc.tensor.matmul(
                            pw[:, lo:lo + 256],
                            ts[:, du * 128:(du + 1) * 128],
                            w_all[:, du * 256:(du + 1) * 256],
                            start=(du % 2 == 0 and du > 0),
                            stop=(du % 2 == 1 or du == DU - 1),
                            skip_group_check=True,
                        )
                    # ---- prefix totals: one reduce over the 3 pre regions ----
                    pre_s = mid.tile([P, P], F32, name="pre_s", tag="pre_s")
                    pre_v = pw[:].rearrange(
                        "p (bk c) -> p bk c", bk=NB
                    )[:, :, 128:256].transpose([0, 2, 1])
                    nc.vector.tensor_reduce(
                        pre_s[:], pre_v,
                        axis=mybir.AxisListType.X, op=mybir.AluOpType.add,
                    )
                    # ---- combine: out[(o, s)] = intra + pre_bcast ----
                    # intra psum layout is (o, v); u = bk2*2 + half
                    out4 = out_sb[:].rearrange(
                        "p (o w bk2 half v) -> w p o bk2 half v",
                        o=O, w=NW, bk2=NU // 2, half=2, v=V
                    )[wdw, :, :, first_u // 2:NU // 2]
                    in1 = pw[:].rearrange(
                        "p (bk half x) -> p bk half x", bk=NB, half=2
                    )[:, :, :, 0:128]
                    in1 = in1.rearrange(
                        "p bk half (o v) -> p o bk half v", v=V, o=O
                    )
                    in2 = pre_s[:].rearrange(
                        "p (bk2 half o) -> p o bk2 half",
                        bk2=NU // 2, half=2, o=O
                    )[:, :, first_u // 2:NU // 2]
                    in2 = in2.unsqueeze(4)
                    in2 = in2.broadcast_to([P, O, NB, 2, V])
                    nc.vector.tensor_add(out4, in1, in2)
                    # last supertile: store the first half as soon as its
                    # windows are done, to shorten the exposed tail
                    if st == NST - 1 and wdw == NW // 2 - 1:
                        out_v = out_sb[:].rearrange("p (o hs) -> p o hs", o=O)
                        nc.scalar.dma_start(yv[st][:, :, 0:H],
                                            out_v[:, :, 0:H])
                if st == NST - 1:
                    out_v = out_sb[:].rearrange("p (o hs) -> p o hs", o=O)
                    nc.scalar.dma_start(yv[st][:, :, H:CH], out_v[:, :, H:CH])
                else:
                    nc.scalar.dma_start(
                        yv[st], out_sb[:].rearrange("p (o hs) -> p o hs", o=O)
                    )
    nc.compile()
    return nc


def _host_constants(weight, bias, n_discard, n_keep):
    S = n_discard + n_keep
    assert S == NU * V
    w = weight.reshape(O, C, n_keep).transpose(2, 1, 0)  # (n_keep, C, O)
    w_full = np.concatenate(
        [np.zeros((n_discard, C, O), np.float32), w.astype(np.float32)], axis=0
    )  # (S, C, O)
    act = [u for u in range(NU)
           if np.abs(w_full[u * V:(u + 1) * V]).max() > 0]
    # kernel assumes active blocks are trailing & contiguous
    first_u = act[0] if act else NU
    assert act == list(range(first_u, NU))
    DU = len(act)
    rhs = np.zeros((DU, P, 256), np.float32)
    vp_idx = np.arange(V)
    for idx, u in enumerate(act):
        blk = w_full[u * V:(u + 1) * V]  # (V, C, O)
        # Wtri: k=(c,vp) -> n=(o,v)
        tri = np.zeros((C, V, V, O), np.float32)
        for v in range(V):
            tri[:, vp_idx <= v, v, :] = blk.transpose(1, 0, 2)[:, vp_idx <= v]
        Wtri = tri.transpose(0, 1, 3, 2).reshape(C * V, O * V)
        # Wpre: k=(c,vp) -> n=(ut,o)
        pre = np.zeros((C, V, NU, O), np.float32)
        for ut in range(NU):
            if ut > u:
                pre[:, :, ut, :] = blk.transpose(1, 0, 2)
        Wpre = pre.reshape(C * V, NU * O)
        if idx % 2 == 0:
            rhs[idx] = np.concatenate([Wtri, Wpre], axis=1)
        else:
            rhs[idx] = np.concatenate([Wpre, Wtri], axis=1)
    w_all = rhs.transpose(1, 0, 2).reshape(P, DU * 256)
    bias32 = bias.astype(np.float32)
    consts = {
        "w_all": np.ascontiguousarray(w_all).astype(ml_dtypes.bfloat16),
        "bias_pre": np.ascontiguousarray(
            np.tile(bias32, NU)[None, :] * np.ones((P, 1), np.float32)
        ),
        "ident": np.eye(P, dtype=np.float32).astype(ml_dtypes.bfloat16),
        "seed": np.concatenate(
            [np.ones(P, np.float32), np.tile(bias32, NU)]
        ).reshape(1, 2 * P).astype(ml_dtypes.bfloat16),
    }
    return consts, DU


def _run(inputs, trace=False):
    x = np.asarray(inputs["x"], dtype=np.float32)
    weight = np.asarray(inputs["weight"], dtype=np.float32)
    bias = np.asarray(inputs["bias"], dtype=np.float32)
    n_discard = int(inputs["n_discard"])
    n_keep = int(inputs["n_keep"])
    assert x.shape == (B, C, T) and weight.shape == (O, C * n_keep)

    consts, DU = _host_constants(weight, bias, n_discard, n_keep)
    key = ("nc", DU)
    if key not in _cache:
        _cache[key] = _build_nc(DU)
    nc = _cache[key]

    in_maps = []
    for b in range(B):
        m = dict(consts)
        m["x"] = np.ascontiguousarray(x[b])
        in_maps.append(m)
    res = run_bass_kernel_spmd(nc, in_maps, list(range(B)), trace=trace)
    y = np.stack([res.results[b]["y"] for b in range(B)], axis=0)
    return y, res


def kernel(**inputs):
    y, _ = _run(inputs, trace=False)
    return y


# revision 13
# speedup vs baseline: 1.0117x; 1.0117x over previous
"""Trainium2 Bass kernel for nn_CumulativeFlattenedLinear (segment_reduce).

Computation: per window of S=64 timesteps, per-timestep C->O linear projection
(weights zero for the first n_discard steps) followed by a causal cumsum within
the window, plus bias.

Strategy (data-parallel over batch, 1 batch element per core):
  - Per 8-step sub-block u: a triangular-masked "intra" matmul plus a "prefix"
    matmul targeting later sub-blocks; both issued as ONE stacked N=256 bf16
    matmul writing [intra | pre] contiguously in PSUM (pre region shared per
    u-pair, accumulated in PSUM).
  - x is loaded with partition = 256-element time chunk (1KB contiguous DMA
    runs, split in two half-tile DMAs so compute starts early), shuffled+cast
    to bf16 in (u, c, v') column order on the Scalar engine, transposed
    128x128 on the TensorEngine (bf16: 1 cyc/row), copied PSUM->SBUF (Scalar).
  - prefix totals summed across the 3 pair-regions + bias (DVE), then one
    strided combine per window writes the (o, t)-ordered output tile; the
    bias-only head region (s < n_discard) is filled once per supertile (DVE).
  - Output stored in two half-tile DMAs per supertile (Scalar queue) so the
    store stream overlaps the input stream; constants load on the GpSimd
    queue so the first x tile is the first packet on the input queue.
"""
import numpy as np
import ml_dtypes

import concourse.bass as bass
import concourse.tile as tile
from concourse import bacc, mybir
from concourse.bass_utils import run_bass_kernel_spmd

F32 = mybir.dt.float32
BF16 = mybir.dt.bfloat16

# problem geometry (asserted against inputs at runtime)
B, C, T, O = 8, 16, 131072, 16
P = 128
CH = 256                 # time-elements per partition per supertile
NST = T // (P * CH)      # 4 supertiles
V = 8                    # sub-block length
NU = 8                   # sub-blocks per window

_cache = {}


def _build_nc(du_count):
    """Build the per-core Bass program. du_count = number of active sub-blocks
    (those with any nonzero weight), assumed to be the trailing ones."""
    S = NU * V  # 64
    NW = CH // S  # windows per partition = 4
    DU = du_count
    first_u = NU - DU          # first active sub-block
    fill_s = first_u * V       # s < fill_s -> output = bias

    nc = bacc.Bacc("TRN2", target_bir_lowering=False, debug=False)
    x_d = nc.dram_tensor("x", (C, T), F32, kind="ExternalInput")
    w_d = nc.dram_tensor("w_all", (P, DU * 256), BF16, kind="ExternalInput")
    bpre_d = nc.dram_tensor("bias_pre", (P, P), F32, kind="ExternalInput")
    ident_d = nc.dram_tensor("ident", (P, P), BF16, kind="ExternalInput")
    seed_d = nc.dram_tensor("seed", (1, 2 * P), BF16, kind="ExternalInput")
    y_d = nc.dram_tensor("y", (O, T), F32, kind="ExternalOutput")

    xv = x_d.ap().rearrange("c (st p hs) -> st p c hs", st=NST, p=P, hs=CH)
    yv = y_d.ap().rearrange("o (st p hs) -> st p o hs", st=NST, p=P, hs=CH)

    NB = (DU + 1) // 2  # psum banks per window group
    H = CH // 2         # half-tile split for load/store DMAs

    with tile.TileContext(nc) as tc:
        with (
            tc.tile_pool(name="const", bufs=1) as cp,
            tc.tile_pool(name="io", bufs=3) as io,
            tc.tile_pool(name="mid", bufs=3) as mid,
            tc.tile_pool(name="psW", bufs=2, space="PSUM") as psW,
            tc.tile_pool(name="psT", bufs=2, space="PSUM") as psT,
        ):
            # constants on the Scalar DMA queue (idle until the first store):
            # the Sync queue's first descriptors are then the first x tile.
            w_all = cp.tile([P, DU * 256], BF16, name="w_all")
            nc.scalar.dma_start(w_all[:], w_d.ap())
            bias_pre = cp.tile([P, P], F32, name="bias_pre")
            nc.scalar.dma_start(bias_pre[:], bpre_d.ap())
            ident = cp.tile([P, P], BF16, name="ident")
            nc.scalar.dma_start(ident[:], ident_d.ap())
            # seed row: [ones(128) | bias tiled over (u,o)] on one partition
            seed = cp.tile([1, 2 * P], BF16, name="seed")
            nc.scalar.dma_start(seed[:], seed_d.ap())

            def back_half(st, wdw, ts, out_sb):
                """Matmuls + reduce + combine + store for one window group.
                Emitted one window late (software pipelining) so the PE never
                waits on the DVE ts-copy of the same window."""
                pw = psW.tile([P, NB * 512], F32, name="pw", tag="pw")
                nc.tensor.matmul(
                    pw[:, 128:256], seed[:, 0:P], seed[:, P:2 * P],
                    start=True, stop=False, skip_group_check=True,
                )
                for du in range(DU):
                    bk = du // 2
                    lo = bk * 512 + (du % 2) * 128
                    nc.tensor.matmul(
                        pw[:, lo:lo + 256],
                        ts[:, du * 128:(du + 1) * 128],
                        w_all[:, du * 256:(du + 1) * 256],
                        start=(du % 2 == 0 and du > 0),
                        stop=(du % 2 == 1 or du == DU - 1),
                        skip_group_check=True,
                    )
                # prefix totals: one reduce over the 3 bias-seeded pre regions
                pre_s = mid.tile([P, P], F32, name="pre_s", tag="pre_s")
                pre_v = pw[:].rearrange(
                    "p (bk c) -> p bk c", bk=NB
                )[:, :, 128:256].transpose([0, 2, 1])
                nc.vector.tensor_reduce(
                    pre_s[:], pre_v,
                    axis=mybir.AxisListType.X, op=mybir.AluOpType.add,
                )
                # combine: out[(o, s)] = intra + pre_bcast; intra is (o, v)
                out4 = out_sb[:].rearrange(
                    "p (o w bk2 half v) -> w p o bk2 half v",
                    o=O, w=NW, bk2=NU // 2, half=2, v=V
                )[wdw, :, :, first_u // 2:NU // 2]
                in1 = pw[:].rearrange(
                    "p (bk half x) -> p bk half x", bk=NB, half=2
                )[:, :, :, 0:128]
                in1 = in1.rearrange(
                    "p bk half (o v) -> p o bk half v", v=V, o=O
                )
                in2 = pre_s[:].rearrange(
                    "p (bk2 half o) -> p o bk2 half",
                    bk2=NU // 2, half=2, o=O
                )[:, :, first_u // 2:NU // 2]
                in2 = in2.unsqueeze(4)
                in2 = in2.broadcast_to([P, O, NB, 2, V])
                nc.vector.tensor_add(out4, in1, in2)
                out_v = out_sb[:].rearrange("p (o hs) -> p o hs", o=O)
                if st == NST - 1 and wdw == NW // 2 - 1:
                    # last supertile: store the first half early
                    nc.scalar.dma_start(yv[st][:, :, 0:H], out_v[:, :, 0:H])
                elif wdw == NW - 1:
                    if st == NST - 1:
                        nc.scalar.dma_start(yv[st][:, :, H:CH],
                                            out_v[:, :, H:CH])
                    else:
                        nc.scalar.dma_start(yv[st], out_v)

            pend = None
            for st in range(NST):
                xin = io.tile([P, C * CH], F32, name="xin", tag="xin")
                if st == 0:
                    # split the first load so window-group 0 starts early
                    xin_v = xin[:].rearrange("p (c hs) -> p c hs", c=C)
                    nc.sync.dma_start(xin_v[:, :, 0:H], xv[st][:, :, 0:H])
                    nc.sync.dma_start(xin_v[:, :, H:CH], xv[st][:, :, H:CH])
                else:
                    nc.sync.dma_start(
                        xin[:].rearrange("p (c hs) -> p c hs", c=C), xv[st]
                    )
                out_sb = io.tile([P, O * CH], F32, name="out_sb", tag="out")
                # ---- bias fill for s < fill_s, all windows of this st ----
                if fill_s:
                    outf = out_sb[:].rearrange(
                        "p (o w u v) -> p o w u v", o=O, w=NW, u=NU, v=V
                    )[:, :, :, 0:first_u]
                    bsrc = (
                        bias_pre[:, 0:O]
                        .unsqueeze(2).unsqueeze(3).unsqueeze(4)
                        .broadcast_to([P, O, NW, first_u, V])
                    )
                    nc.gpsimd.tensor_copy(outf, bsrc)
                for wdw in range(NW):
                    # ---- shuffle+cast to (du, c, v) bf16 (Scalar) ----
                    shuf = mid.tile([P, DU * 128], BF16, name="shuf", tag="shuf")
                    src = xin[:].rearrange(
                        "p (c w u v) -> w p u c v", c=C, w=NW, u=NU, v=V
                    )[wdw, :, first_u:NU]
                    nc.scalar.copy(
                        shuf[:].rearrange("p (u c v) -> p u c v", u=DU, c=C, v=V),
                        src,
                    )
                    # ---- transposes (PE): all DU into one bf16 psum bank ----
                    pt = psT.tile([P, DU * 128], BF16, name="pt", tag="pt")
                    for j in range(DU):
                        nc.tensor.transpose(
                            pt[:, j * 128:(j + 1) * 128],
                            shuf[:, j * 128:(j + 1) * 128],
                            ident[:],
                            tile_position=(0, 0),
                        )
                    ts = mid.tile([P, DU * 128], BF16, name="ts", tag="ts")
                    nc.vector.tensor_copy(ts[:], pt[:])
                    if pend is not None:
                        back_half(*pend)
                    pend = (st, wdw, ts, out_sb)
            back_half(*pend)
    nc.compile()
    return nc


def _host_constants(weight, bias, n_discard, n_keep):
    S = n_discard + n_keep
    assert S == NU * V
    w = weight.reshape(O, C, n_keep).transpose(2, 1, 0)  # (n_keep, C, O)
    w_full = np.concatenate(
        [np.zeros((n_discard, C, O), np.float32), w.astype(np.float32)], axis=0
    )  # (S, C, O)
    act = [u for u in range(NU)
           if np.abs(w_full[u * V:(u + 1) * V]).max() > 0]
    # kernel assumes active blocks are trailing & contiguous
    first_u = act[0] if act else NU
    assert act == list(range(first_u, NU))
    DU = len(act)
    rhs = np.zeros((DU, P, 256), np.float32)
    vp_idx = np.arange(V)
    for idx, u in enumerate(act):
        blk = w_full[u * V:(u + 1) * V]  # (V, C, O)
        # Wtri: k=(c,vp) -> n=(o,v)
        tri = np.zeros((C, V, V, O), np.float32)
        for v in range(V):
            tri[:, vp_idx <= v, v, :] = blk.transpose(1, 0, 2)[:, vp_idx <= v]
        Wtri = tri.transpose(0, 1, 3, 2).reshape(C * V, O * V)
        # Wpre: k=(c,vp) -> n=(ut,o)
        pre = np.zeros((C, V, NU, O), np.float32)
        for ut in range(NU):
            if ut > u:
                pre[:, :, ut, :] = blk.transpose(1, 0, 2)
        Wpre = pre.reshape(C * V, NU * O)
        if idx % 2 == 0:
            rhs[idx] = np.concatenate([Wtri, Wpre], axis=1)
        else:
            rhs[idx] = np.concatenate([Wpre, Wtri], axis=1)
    w_all = rhs.transpose(1, 0, 2).reshape(P, DU * 256)
    bias32 = bias.astype(np.float32)
    consts = {
        "w_all": np.ascontiguousarray(w_all).astype(ml_dtypes.bfloat16),
        "bias_pre": np.ascontiguousarray(
            np.tile(bias32, NU)[None, :] * np.ones((P, 1), np.float32)
        ),
        "ident": np.eye(P, dtype=np.float32).astype(ml_dtypes.bfloat16),
        "seed": np.concatenate(
            [np.ones(P, np.float32), np.tile(bias32, NU)]
        ).reshape(1, 2 * P).astype(ml_dtypes.bfloat16),
    }
    return consts, DU


def _run(inputs, trace=False):
    x = np.asarray(inputs["x"], dtype=np.float32)
    weight = np.asarray(inputs["weight"], dtype=np.float32)
    bias = np.asarray(inputs["bias"], dtype=np.float32)
    n_discard = int(inputs["n_discard"])
    n_keep = int(inputs["n_keep"])
    assert x.shape == (B, C, T) and weight.shape == (O, C * n_keep)

    consts, DU = _host_constants(weight, bias, n_discard, n_keep)
    key = ("nc", DU)
    if key not in _cache:
        _cache[key] = _build_nc(DU)
    nc = _cache[key]

    in_maps = []
    for b in range(B):
        m = dict(consts)
        m["x"] = np.ascontiguousarray(x[b])
        in_maps.append(m)
    res = run_bass_kernel_spmd(nc, in_maps, list(range(B)), trace=trace)
    y = np.stack([res.results[b]["y"] for b in range(B)], axis=0)
    return y, res


def kernel(**inputs):
    y, _ = _run(inputs, trace=False)
    return y
